# revision 3
# baseline (speedup 1.0000x reference)
"""GQA + ALiBi causal attention on 8 Trainium2 NeuronCores (Bass/Tile).

Sharding: tensor-parallel over the 8 KV head groups (core c handles query
heads 4c..4c+3, which share KV head c).  Each core runs a specialized Bass
program (per-head ALiBi slopes are folded into compile-time schedules), all 8
dispatched concurrently via PJRT, one per NeuronCore.

Per-core algorithm (S^T blocked attention, no max pass):
  scores tile  psum[j=128, i<=512] = ktile^T @ qchunk        (float32r matmul)
  texp = exp(psum + bias[j])  on ACT, bf16 out; bias = slope*(j - ref_chunk)
         (softmax shift-invariance makes the per-chunk ref exact; chunk width
          is capped per head so exp never over/underflows)
  diagonal tiles: texp *= triangle mask (DVE bf16)
  PV:  psum_acc[i=128, 136] += texp^T @ [v | ones]           (bf16 matmul)
  out = acc[:, :128] * reciprocal(acc[:, 128])               (DVE)
ALiBi windowing: key tiles more than ~60/slope behind the oldest query of a
chunk are skipped (their weights underflow to 0 in fp32 anyway).
"""

import json
import math
from contextlib import ExitStack

import numpy as np
import ml_dtypes

B, S, H, KV, D = 1, 2048, 32, 8, 128
G = H // KV
VF = 136  # v columns (128) + ones (1) + pad
JT = 128
NJT = S // JT

# ----------------------------------------------------------------------------
# BIR fix: this walrus build accepts only ONE sync-wait per instruction.
# Hoist excess waits onto preceding single-wait NoOps (same engine => program
# order preserved).  Installed as a Bass.to_json_bytes wrap.
# ----------------------------------------------------------------------------


def _fix_sync_waits(bir_bytes: bytes) -> bytes:
    m = json.loads(bir_bytes)
    changed = False
    for f in m.get("functions", []):
        for b in f.get("blocks", []):
            new_list = []
            for ins in b.get("instructions", []):
                si = ins.get("sync_info") or {}
                waits = si.get("on_wait") or []
                if len(waits) <= 1:
                    new_list.append(ins)
                    continue
                changed = True
                for ci, w in enumerate(waits[:-1]):
                    new_list.append(
                        {
                            "debug": ins.get("debug", 0),
                            "engine": ins["engine"],
                            "ins": [],
                            "name": f"{ins['name']}hw{ci}",
                            "opcode": "NoOp",
                            "outs": [],
                            "sync_info": {"on_wait": [w], "on_update": []},
                        }
                    )
                d = dict(ins)
                d["sync_info"] = {
                    "on_wait": waits[-1:],
                    "on_update": si.get("on_update", []),
                }
                new_list.append(d)
            b["instructions"] = new_list
    return json.dumps(m).encode() if changed else bir_bytes


def _install_bir_fix():
    import concourse.bass as bass

    if getattr(bass.Bass, "_drainfix_installed", False):
        return
    orig = bass.Bass.to_json_bytes

    def to_json_bytes(self, *a, **kw):
        return _fix_sync_waits(orig(self, *a, **kw))

    bass.Bass.to_json_bytes = to_json_bytes
    bass.Bass._drainfix_installed = True


# ----------------------------------------------------------------------------
# Per-head schedule
# ----------------------------------------------------------------------------


def _chunk_width(slope: float) -> int:
    a = abs(slope)
    for C in (512, 256, 128, 64):
        if a * (C - 1) <= 65.0:
            return C
    return 64


def _window_keys(slope: float, threshold: float = 60.0) -> int:
    if slope <= 0.0:
        return S
    return min(S, int(math.ceil(threshold / slope)))


def _head_plan(slope: float):
    C = _chunk_width(slope)
    win = _window_keys(slope)
    chunks = []
    for ic in range(S // C):
        i_start = ic * C
        i_end = i_start + C - 1
        ref = i_end
        jt_hi = i_end // JT
        jt_lo = max(0, (i_start - win) // JT) if win < S else 0
        nsub = max(1, C // 128)
        sub_rows = min(128, C)
        sub_first = {s: jt_lo for s in range(nsub)}
        sub_last = {
            s: min(jt_hi, (i_start + s * 128 + sub_rows - 1) // JT)
            for s in range(nsub)
        }
        jts = []
        for jt in range(jt_lo, jt_hi + 1):
            o = jt * JT - i_start
            col_start = max(0, o) if C >= 128 else 0
            cols = C - col_start
            r_bias = jt * JT - ref
            shift = i_start + col_start - jt * JT
            mask_shift = shift if shift < 128 else None
            subs = []
            for sub in range(nsub):
                sub_i0 = i_start + sub * 128
                if jt * JT > sub_i0 + sub_rows - 1 or jt < sub_first[sub]:
                    continue
                subs.append(
                    (
                        sub,
                        sub_i0 - i_start - col_start,
                        sub_rows,
                        jt == sub_first[sub],
                        jt == sub_last[sub],
                    )
                )
            jts.append((jt, col_start, cols, r_bias, mask_shift, subs))
        chunks.append((i_start, C, ref, jts))
    return {"C": C, "win": win, "chunks": chunks}


def _index_maps(plans):
    bias_idx, nb = {}, 0
    for h, plan in enumerate(plans):
        rs = set()
        for (_i0, _C, _ref, jts) in plan["chunks"]:
            for (_jt, _cs, _cols, r, _ms, _subs) in jts:
                rs.add(r)
        for r in sorted(rs):
            bias_idx[(h, r)] = nb
            nb += 1
    shifts = set()
    for plan in plans:
        for (_i0, _C, _ref, jts) in plan["chunks"]:
            for (_jt, _cs, _cols, _r, ms, _subs) in jts:
                if ms is not None:
                    shifts.add(ms)
    mask_idx = {s: i for i, s in enumerate(sorted(shifts))}
    return bias_idx, nb, mask_idx, max(1, len(shifts))


# ----------------------------------------------------------------------------
# Bass program builder (one core)
# ----------------------------------------------------------------------------


def _build_core_program(slopes, qk_bufs=3, texp_bufs=3):
    import concourse.bass as bass
    import concourse.mybir as mybir
    import concourse.tile as tile

    F32 = mybir.dt.float32
    F32R = mybir.dt.float32r
    BF16 = mybir.dt.bfloat16

    NH = len(slopes)
    plans = [_head_plan(s) for s in slopes]
    bias_idx, nb, mask_idx, NM = _index_maps(plans)

    nc = bass.Bass(target_bir_lowering=False, trn_type="TRN2", debug=False)
    qt_d = nc.dram_tensor("qt", [128, NH * S], F32R, kind="ExternalInput").ap()
    kt_d = nc.dram_tensor("kt", [128, S], F32R, kind="ExternalInput").ap()
    vx_d = nc.dram_tensor("vx", [128, NJT * VF], BF16, kind="ExternalInput").ap()
    bias_d = nc.dram_tensor("bias", [128, nb], F32, kind="ExternalInput").ap()
    msk_d = nc.dram_tensor("msk", [128, NM * 128], BF16, kind="ExternalInput").ap()
    out_d = nc.dram_tensor("out", [NH, S, D], F32, kind="ExternalOutput").ap()

    with tile.TileContext(nc) as tc, ExitStack() as ctx:
        consts = ctx.enter_context(tc.tile_pool(name="consts", bufs=1))
        qkp = ctx.enter_context(tc.tile_pool(name="qk", bufs=qk_bufs, space="PSUM"))
        accp = ctx.enter_context(tc.tile_pool(name="acc", bufs=1, space="PSUM"))
        tep = ctx.enter_context(tc.tile_pool(name="texp", bufs=texp_bufs))
        outp = ctx.enter_context(tc.tile_pool(name="outs", bufs=2))
        smallp = ctx.enter_context(tc.tile_pool(name="small", bufs=2))

        kt_sb = consts.tile([128, S], F32R, name="kt_sb")
        nc.sync.dma_start(kt_sb[:], kt_d)
        vx_sb = consts.tile([128, NJT * VF], BF16, name="vx_sb")
        nc.sync.dma_start(vx_sb[:], vx_d)
        bias_sb = consts.tile([128, nb], F32, name="bias_sb")
        nc.sync.dma_start(bias_sb[:], bias_d)
        msk_sb = consts.tile([128, NM * 128], BF16, name="msk_sb")
        nc.sync.dma_start(msk_sb[:], msk_d)
        qt_sb = []
        for h in range(NH):
            t = consts.tile([128, S], F32R, tag=f"qt{h}", name=f"qt{h}")
            nc.sync.dma_start(t[:], qt_d[:, h * S : (h + 1) * S])
            qt_sb.append(t)

        for h in range(NH):
            plan = plans[h]
            C = plan["C"]
            nsub = max(1, C // 128)
            sub_rows = min(128, C)
            for (i_start, _C, _ref, jts) in plan["chunks"]:
                accs = [
                    accp.tile([sub_rows, VF], mybir.dt.float32, tag=f"acc{s}", name=f"acc{s}")
                    for s in range(nsub)
                ]
                for (jt, col_start, cols, r_bias, mask_shift, subs) in jts:
                    qk = qkp.tile([128, C], mybir.dt.float32, tag="qk", name="qk")
                    nc.tensor.matmul(
                        qk[:, :cols],
                        lhsT=kt_sb[:, jt * JT : (jt + 1) * JT],
                        rhs=qt_sb[h][:, i_start + col_start : i_start + C],
                        start=True,
                        stop=True,
                    )
                    texp = tep.tile([128, C], BF16, tag="texp", name="texp")
                    bi = bias_idx[(h, r_bias)]
                    nc.scalar.activation(
                        texp[:, :cols],
                        qk[:, :cols],
                        mybir.ActivationFunctionType.Exp,
                        bias=bias_sb[:, bi : bi + 1],
                    )
                    if mask_shift is not None:
                        mi = mask_idx[mask_shift]
                        mw = min(128, cols)
                        nc.vector.tensor_tensor(
                            texp[:, :mw],
                            texp[:, :mw],
                            msk_sb[:, mi * 128 : mi * 128 + mw],
                            mybir.AluOpType.mult,
                        )
                    for (sub, local, srows, first, last) in subs:
                        nc.tensor.matmul(
                            accs[sub][:, :],
                            lhsT=texp[:, local : local + srows],
                            rhs=vx_sb[:, jt * VF : (jt + 1) * VF],
                            start=first,
                            stop=last,
                        )
                out_sb = outp.tile(
                    [sub_rows, nsub * D], mybir.dt.float32, tag="out_sb", name="out_sb"
                )
                for sidx in range(nsub):
                    rec = smallp.tile([sub_rows, 1], mybir.dt.float32, tag="rec", name="rec")
                    nc.vector.reciprocal(rec[:], accs[sidx][:, D : D + 1])
                    nc.vector.tensor_tensor(
                        out_sb[:, sidx * D : (sidx + 1) * D],
                        accs[sidx][:, :D],
                        rec[:, 0:1].to_broadcast((sub_rows, D)),
                        mybir.AluOpType.mult,
                    )
                nc.sync.dma_start(
                    out_d[h, i_start : i_start + C, :].rearrange(
                        "(n p) d -> p n d", p=sub_rows
                    ),
                    out_sb[:].rearrange("p (n d) -> p n d", d=D),
                )

    return nc, plans, bias_idx, mask_idx, NM, nb


# ----------------------------------------------------------------------------
# Host-side input packing (must mirror the builder's index maps)
# ----------------------------------------------------------------------------


def _build_host_inputs(q_heads, k, v, slopes, plans, bias_idx, mask_idx, NM, nb):
    NH = len(q_heads)
    SCALE = 1.0 / math.sqrt(D)
    qt = np.empty((128, NH * S), np.float32)
    for h in range(NH):
        qt[:, h * S : (h + 1) * S] = q_heads[h].T * SCALE
    kt = np.ascontiguousarray(k.T.astype(np.float32))
    vext = np.zeros((S, VF), np.float32)
    vext[:, :D] = v
    vext[:, D] = 1.0
    vx = np.empty((128, NJT * VF), ml_dtypes.bfloat16)
    for jt in range(NJT):
        vx[:, jt * VF : (jt + 1) * VF] = vext[jt * 128 : (jt + 1) * 128].astype(
            ml_dtypes.bfloat16
        )
    bias = np.zeros((128, nb), np.float32)
    for (h, r), idx in bias_idx.items():
        bias[:, idx] = slopes[h] * (np.arange(128, dtype=np.float64) + r)
    msk = np.zeros((128, NM * 128), ml_dtypes.bfloat16)
    p = np.arange(128)[:, None]
    f = np.arange(128)[None, :]
    for sft, i in mask_idx.items():
        msk[:, i * 128 : (i + 1) * 128] = (p <= f + sft).astype(ml_dtypes.bfloat16)
    return {"qt": qt, "kt": kt, "vx": vx, "bias": bias, "msk": msk}


# ----------------------------------------------------------------------------
# PJRT per-device dispatch (mirrors bass2jax.run_bass_via_pjrt n_cores==1,
# but pinned to a chosen device so the 8 specialized programs overlap)
# ----------------------------------------------------------------------------

_CORE_CACHE = {}


def _make_core_runner(slopes_key, slopes, device):
    import jax
    import concourse.mybir as mybir
    from concourse import bass2jax

    bass2jax.install_neuronx_cc_hook()
    nc, plans, bias_idx, mask_idx, NM, nb = _build_core_program(list(slopes))

    partition_name = nc.partition_id_tensor.name if nc.partition_id_tensor else None
    in_names, out_names, out_avals, zero_outs = [], [], [], []
    for alloc in nc.m.functions[0].allocations:
        if not isinstance(alloc, mybir.MemoryLocationSet):
            continue
        name = alloc.memorylocations[0].name
        if alloc.kind == "ExternalInput":
            if name != partition_name:
                in_names.append(name)
        elif alloc.kind == "ExternalOutput":
            out_names.append(name)
            shape = tuple(alloc.tensor_shape)
            dtype = mybir.dt.np(alloc.dtype)
            out_avals.append(jax.core.ShapedArray(shape, dtype))
            zero_outs.append(np.zeros(shape, dtype))
    n_params = len(in_names)
    all_names = in_names + out_names
    if partition_name is not None:
        all_names = all_names + [partition_name]
    donate = tuple(range(n_params, n_params + len(out_names)))

    def _body(*args):
        operands = list(args)
        if partition_name is not None:
            operands.append(bass2jax.partition_id_tensor())
        outs = bass2jax._bass_exec_p.bind(
            *operands,
            out_avals=tuple(out_avals),
            in_names=tuple(all_names),
            out_names=tuple(out_names),
            lowering_input_output_aliases=(),
            sim_require_finite=True,
            sim_require_nnan=True,
            nc=nc,
        )
        return tuple(outs)

    jitted = jax.jit(_body, donate_argnums=donate, keep_unused=True)

    meta = (plans, bias_idx, mask_idx, NM, nb)

    def run(host_arrays):
        import jax

        args = [jax.device_put(host_arrays[n], device) for n in in_names]
        args += [jax.device_put(z, device) for z in zero_outs]
        outs = jitted(*args)
        return dict(zip(out_names, outs))

    return run, meta, list(slopes)


def kernel(query, key, value, alibi_slopes):
    import jax

    _install_bir_fix()
    query = np.asarray(query, np.float32)
    key = np.asarray(key, np.float32)
    value = np.asarray(value, np.float32)
    slopes_all = np.asarray(alibi_slopes, np.float32)

    devices = jax.devices()[:KV]
    b = query.shape[0]
    assert b == B and query.shape[1] == S

    # build/cache per-core runners
    runners = []
    for c in range(KV):
        sl = tuple(float(x) for x in slopes_all[c * G : (c + 1) * G])
        ck = (c, sl)
        if ck not in _CORE_CACHE:
            _CORE_CACHE[ck] = _make_core_runner(ck, sl, devices[c])
        runners.append(_CORE_CACHE[ck])

    q = query.reshape(S, H, D)  # B==1
    kf = key.reshape(S, KV, D)
    vf = value.reshape(S, KV, D)

    # dispatch all cores (async), then gather
    pending = []
    for c in range(KV):
        run, meta, sl = runners[c]
        plans, bias_idx, mask_idx, NM, nb = meta
        q_heads = np.ascontiguousarray(
            q[:, c * G : (c + 1) * G, :].transpose(1, 0, 2)
        )
        hi = _build_host_inputs(
            q_heads, kf[:, c, :], vf[:, c, :], sl, plans, bias_idx, mask_idx, NM, nb
        )
        pending.append(run(hi))

    out = np.empty((B, S, H * D), np.float32)
    for c in range(KV):
        oc = np.asarray(pending[c]["out"])  # [G, S, D]
        for g in range(G):
            h = c * G + g
            out[0, :, h * D : (h + 1) * D] = oc[g]
    return out


# revision 4
# speedup vs baseline: 1.0503x; 1.0503x over previous
"""GQA + ALiBi causal attention on 8 Trainium2 NeuronCores (Bass/Tile).

Sharding: tensor-parallel over the 8 KV head groups (core c handles query
heads 4c..4c+3, which share KV head c).  Each core runs a specialized Bass
program (per-head ALiBi slopes are folded into compile-time schedules), all 8
dispatched concurrently via PJRT, one per NeuronCore.

Per-core algorithm (S^T blocked attention, no max pass):
  scores tile  psum[j=128, i<=512] = ktile^T @ qchunk        (float32r matmul)
  texp = exp(psum + bias[j])  on ACT, bf16 out; bias = slope*(j - ref_chunk)
         (softmax shift-invariance makes the per-chunk ref exact; chunk width
          is capped per head so exp never over/underflows)
  diagonal tiles: texp *= triangle mask (DVE bf16)
  PV:  psum_acc[i=128, 136] += texp^T @ [v | ones]           (bf16 matmul)
  out = acc[:, :128] * reciprocal(acc[:, 128])               (DVE)
ALiBi windowing: key tiles more than ~60/slope behind the oldest query of a
chunk are skipped (their weights underflow to 0 in fp32 anyway).
"""

import json
import math
from contextlib import ExitStack

import numpy as np
import ml_dtypes

B, S, H, KV, D = 1, 2048, 32, 8, 128
G = H // KV
VF = 136  # v columns (128) + ones (1) + pad
JT = 128
NJT = S // JT

# ----------------------------------------------------------------------------
# BIR fix: this walrus build accepts only ONE sync-wait per instruction.
# Hoist excess waits onto preceding single-wait NoOps (same engine => program
# order preserved).  Installed as a Bass.to_json_bytes wrap.
# ----------------------------------------------------------------------------


def _fix_sync_waits(bir_bytes: bytes) -> bytes:
    m = json.loads(bir_bytes)
    changed = False
    for f in m.get("functions", []):
        for b in f.get("blocks", []):
            new_list = []
            for ins in b.get("instructions", []):
                si = ins.get("sync_info") or {}
                waits = si.get("on_wait") or []
                if len(waits) <= 1:
                    new_list.append(ins)
                    continue
                changed = True
                for ci, w in enumerate(waits[:-1]):
                    new_list.append(
                        {
                            "debug": ins.get("debug", 0),
                            "engine": ins["engine"],
                            "ins": [],
                            "name": f"{ins['name']}hw{ci}",
                            "opcode": "NoOp",
                            "outs": [],
                            "sync_info": {"on_wait": [w], "on_update": []},
                        }
                    )
                d = dict(ins)
                d["sync_info"] = {
                    "on_wait": waits[-1:],
                    "on_update": si.get("on_update", []),
                }
                new_list.append(d)
            b["instructions"] = new_list
    return json.dumps(m).encode() if changed else bir_bytes


def _install_bir_fix():
    import concourse.bass as bass

    if getattr(bass.Bass, "_drainfix_installed", False):
        return
    orig = bass.Bass.to_json_bytes

    def to_json_bytes(self, *a, **kw):
        return _fix_sync_waits(orig(self, *a, **kw))

    bass.Bass.to_json_bytes = to_json_bytes
    bass.Bass._drainfix_installed = True


# ----------------------------------------------------------------------------
# Per-head schedule
# ----------------------------------------------------------------------------


def _chunk_width(slope: float) -> int:
    a = abs(slope)
    for C in (512, 256, 128, 64):
        if a * (C - 1) <= 65.0:
            return C
    return 64


def _window_keys(slope: float, threshold: float = 60.0) -> int:
    if slope <= 0.0:
        return S
    return min(S, int(math.ceil(threshold / slope)))


def _head_plan(slope: float):
    C = _chunk_width(slope)
    win = _window_keys(slope)
    chunks = []
    for ic in range(S // C):
        i_start = ic * C
        i_end = i_start + C - 1
        ref = i_end
        jt_hi = i_end // JT
        jt_lo = max(0, (i_start - win) // JT) if win < S else 0
        nsub = max(1, C // 128)
        sub_rows = min(128, C)
        sub_first = {s: jt_lo for s in range(nsub)}
        sub_last = {
            s: min(jt_hi, (i_start + s * 128 + sub_rows - 1) // JT)
            for s in range(nsub)
        }
        jts = []
        for jt in range(jt_lo, jt_hi + 1):
            o = jt * JT - i_start
            col_start = max(0, o) if C >= 128 else 0
            cols = C - col_start
            r_bias = jt * JT - ref
            shift = i_start + col_start - jt * JT
            mask_shift = shift if shift < 128 else None
            subs = []
            for sub in range(nsub):
                sub_i0 = i_start + sub * 128
                if jt * JT > sub_i0 + sub_rows - 1 or jt < sub_first[sub]:
                    continue
                subs.append(
                    (
                        sub,
                        sub_i0 - i_start - col_start,
                        sub_rows,
                        jt == sub_first[sub],
                        jt == sub_last[sub],
                    )
                )
            jts.append((jt, col_start, cols, r_bias, mask_shift, subs))
        chunks.append((i_start, C, ref, jts))
    return {"C": C, "win": win, "chunks": chunks}


def _index_maps(plans):
    bias_idx, nb = {}, 0
    for h, plan in enumerate(plans):
        rs = set()
        for (_i0, _C, _ref, jts) in plan["chunks"]:
            for (_jt, _cs, _cols, r, _ms, _subs) in jts:
                rs.add(r)
        for r in sorted(rs):
            bias_idx[(h, r)] = nb
            nb += 1
    shifts = set()
    for plan in plans:
        for (_i0, _C, _ref, jts) in plan["chunks"]:
            for (_jt, _cs, _cols, _r, ms, _subs) in jts:
                if ms is not None:
                    shifts.add(ms)
    mask_idx = {s: i for i, s in enumerate(sorted(shifts))}
    return bias_idx, nb, mask_idx, max(1, len(shifts))


# ----------------------------------------------------------------------------
# Bass program builder (one core)
# ----------------------------------------------------------------------------


def _build_core_program(slopes, qk_bufs=3, texp_bufs=4):
    import concourse.bass as bass
    import concourse.mybir as mybir
    import concourse.tile as tile

    F32 = mybir.dt.float32
    F32R = mybir.dt.float32r
    BF16 = mybir.dt.bfloat16

    NH = len(slopes)
    plans = [_head_plan(s) for s in slopes]
    bias_idx, nb, mask_idx, NM = _index_maps(plans)

    nc = bass.Bass(target_bir_lowering=False, trn_type="TRN2", debug=False)
    qt_d = nc.dram_tensor("qt", [128, NH * S], F32R, kind="ExternalInput").ap()
    kt_d = nc.dram_tensor("kt", [128, S], F32R, kind="ExternalInput").ap()
    vx_d = nc.dram_tensor("vx", [128, NJT * VF], BF16, kind="ExternalInput").ap()
    bias_d = nc.dram_tensor("bias", [128, nb], F32, kind="ExternalInput").ap()
    msk_d = nc.dram_tensor("msk", [128, NM * 128], BF16, kind="ExternalInput").ap()
    out_d = nc.dram_tensor("out", [NH, S, D], F32, kind="ExternalOutput").ap()

    with tile.TileContext(nc) as tc, ExitStack() as ctx:
        consts = ctx.enter_context(tc.tile_pool(name="consts", bufs=1))
        max_nsub = max(max(1, _head_plan(s0)["C"] // 128) for s0 in slopes)
        acc_bufs = 2 if max_nsub * 2 + qk_bufs <= 8 else 1
        qkp = ctx.enter_context(tc.tile_pool(name="qk", bufs=qk_bufs, space="PSUM"))
        accp = ctx.enter_context(tc.tile_pool(name="acc", bufs=acc_bufs, space="PSUM"))
        tep = ctx.enter_context(tc.tile_pool(name="texp", bufs=texp_bufs))
        outp = ctx.enter_context(tc.tile_pool(name="outs", bufs=2))
        smallp = ctx.enter_context(tc.tile_pool(name="small", bufs=2))

        kt_sb = consts.tile([128, S], F32R, name="kt_sb")
        nc.sync.dma_start(kt_sb[:], kt_d)
        vx_sb = consts.tile([128, NJT * VF], BF16, name="vx_sb")
        nc.sync.dma_start(vx_sb[:], vx_d)
        bias_sb = consts.tile([128, nb], F32, name="bias_sb")
        nc.sync.dma_start(bias_sb[:], bias_d)
        msk_sb = consts.tile([128, NM * 128], BF16, name="msk_sb")
        nc.sync.dma_start(msk_sb[:], msk_d)
        qt_sb = []
        for h in range(NH):
            t = consts.tile([128, S], F32R, tag=f"qt{h}", name=f"qt{h}")
            nc.sync.dma_start(t[:], qt_d[:, h * S : (h + 1) * S])
            qt_sb.append(t)

        for h in range(NH):
            plan = plans[h]
            C = plan["C"]
            nsub = max(1, C // 128)
            sub_rows = min(128, C)
            for (i_start, _C, _ref, jts) in plan["chunks"]:
                accs = [
                    accp.tile([sub_rows, VF], mybir.dt.float32, tag=f"acc{s}", name=f"acc{s}")
                    for s in range(nsub)
                ]
                for (jt, col_start, cols, r_bias, mask_shift, subs) in jts:
                    qk = qkp.tile([128, C], mybir.dt.float32, tag="qk", name="qk")
                    nc.tensor.matmul(
                        qk[:, :cols],
                        lhsT=kt_sb[:, jt * JT : (jt + 1) * JT],
                        rhs=qt_sb[h][:, i_start + col_start : i_start + C],
                        start=True,
                        stop=True,
                    )
                    texp = tep.tile([128, C], BF16, tag="texp", name="texp")
                    bi = bias_idx[(h, r_bias)]
                    nc.scalar.activation(
                        texp[:, :cols],
                        qk[:, :cols],
                        mybir.ActivationFunctionType.Exp,
                        bias=bias_sb[:, bi : bi + 1],
                    )
                    if mask_shift is not None:
                        mi = mask_idx[mask_shift]
                        mw = min(128, cols)
                        nc.vector.tensor_tensor(
                            texp[:, :mw],
                            texp[:, :mw],
                            msk_sb[:, mi * 128 : mi * 128 + mw],
                            mybir.AluOpType.mult,
                        )
                    for (sub, local, srows, first, last) in subs:
                        nc.tensor.matmul(
                            accs[sub][:, :],
                            lhsT=texp[:, local : local + srows],
                            rhs=vx_sb[:, jt * VF : (jt + 1) * VF],
                            start=first,
                            stop=last,
                        )
                out_sb = outp.tile(
                    [sub_rows, nsub * D], mybir.dt.float32, tag="out_sb", name="out_sb"
                )
                for sidx in range(nsub):
                    rec = smallp.tile([sub_rows, 1], mybir.dt.float32, tag="rec", name="rec")
                    nc.vector.reciprocal(rec[:], accs[sidx][:, D : D + 1])
                    nc.vector.tensor_tensor(
                        out_sb[:, sidx * D : (sidx + 1) * D],
                        accs[sidx][:, :D],
                        rec[:, 0:1].to_broadcast((sub_rows, D)),
                        mybir.AluOpType.mult,
                    )
                nc.sync.dma_start(
                    out_d[h, i_start : i_start + C, :].rearrange(
                        "(n p) d -> p n d", p=sub_rows
                    ),
                    out_sb[:].rearrange("p (n d) -> p n d", d=D),
                )

    return nc, plans, bias_idx, mask_idx, NM, nb


# ----------------------------------------------------------------------------
# Host-side input packing (must mirror the builder's index maps)
# ----------------------------------------------------------------------------


def _build_host_inputs(q_heads, k, v, slopes, plans, bias_idx, mask_idx, NM, nb):
    NH = len(q_heads)
    SCALE = 1.0 / math.sqrt(D)
    qt = np.empty((128, NH * S), np.float32)
    for h in range(NH):
        qt[:, h * S : (h + 1) * S] = q_heads[h].T * SCALE
    kt = np.ascontiguousarray(k.T.astype(np.float32))
    vext = np.zeros((S, VF), np.float32)
    vext[:, :D] = v
    vext[:, D] = 1.0
    vx = np.empty((128, NJT * VF), ml_dtypes.bfloat16)
    for jt in range(NJT):
        vx[:, jt * VF : (jt + 1) * VF] = vext[jt * 128 : (jt + 1) * 128].astype(
            ml_dtypes.bfloat16
        )
    bias = np.zeros((128, nb), np.float32)
    for (h, r), idx in bias_idx.items():
        bias[:, idx] = slopes[h] * (np.arange(128, dtype=np.float64) + r)
    msk = np.zeros((128, NM * 128), ml_dtypes.bfloat16)
    p = np.arange(128)[:, None]
    f = np.arange(128)[None, :]
    for sft, i in mask_idx.items():
        msk[:, i * 128 : (i + 1) * 128] = (p <= f + sft).astype(ml_dtypes.bfloat16)
    return {"qt": qt, "kt": kt, "vx": vx, "bias": bias, "msk": msk}


# ----------------------------------------------------------------------------
# PJRT per-device dispatch (mirrors bass2jax.run_bass_via_pjrt n_cores==1,
# but pinned to a chosen device so the 8 specialized programs overlap)
# ----------------------------------------------------------------------------

_CORE_CACHE = {}


def _make_core_runner(slopes_key, slopes, device):
    import jax
    import concourse.mybir as mybir
    from concourse import bass2jax

    bass2jax.install_neuronx_cc_hook()
    nc, plans, bias_idx, mask_idx, NM, nb = _build_core_program(list(slopes))

    partition_name = nc.partition_id_tensor.name if nc.partition_id_tensor else None
    in_names, out_names, out_avals, zero_outs = [], [], [], []
    for alloc in nc.m.functions[0].allocations:
        if not isinstance(alloc, mybir.MemoryLocationSet):
            continue
        name = alloc.memorylocations[0].name
        if alloc.kind == "ExternalInput":
            if name != partition_name:
                in_names.append(name)
        elif alloc.kind == "ExternalOutput":
            out_names.append(name)
            shape = tuple(alloc.tensor_shape)
            dtype = mybir.dt.np(alloc.dtype)
            out_avals.append(jax.core.ShapedArray(shape, dtype))
            zero_outs.append(np.zeros(shape, dtype))
    n_params = len(in_names)
    all_names = in_names + out_names
    if partition_name is not None:
        all_names = all_names + [partition_name]
    donate = tuple(range(n_params, n_params + len(out_names)))

    def _body(*args):
        operands = list(args)
        if partition_name is not None:
            operands.append(bass2jax.partition_id_tensor())
        outs = bass2jax._bass_exec_p.bind(
            *operands,
            out_avals=tuple(out_avals),
            in_names=tuple(all_names),
            out_names=tuple(out_names),
            lowering_input_output_aliases=(),
            sim_require_finite=True,
            sim_require_nnan=True,
            nc=nc,
        )
        return tuple(outs)

    jitted = jax.jit(_body, donate_argnums=donate, keep_unused=True)

    meta = (plans, bias_idx, mask_idx, NM, nb)

    def run(host_arrays):
        import jax

        args = [jax.device_put(host_arrays[n], device) for n in in_names]
        args += [jax.device_put(z, device) for z in zero_outs]
        outs = jitted(*args)
        return dict(zip(out_names, outs))

    return run, meta, list(slopes)


def kernel(query, key, value, alibi_slopes):
    import jax

    _install_bir_fix()
    query = np.asarray(query, np.float32)
    key = np.asarray(key, np.float32)
    value = np.asarray(value, np.float32)
    slopes_all = np.asarray(alibi_slopes, np.float32)

    devices = jax.devices()[:KV]
    b = query.shape[0]
    assert b == B and query.shape[1] == S

    # build/cache per-core runners
    runners = []
    for c in range(KV):
        sl = tuple(float(x) for x in slopes_all[c * G : (c + 1) * G])
        ck = (c, sl)
        if ck not in _CORE_CACHE:
            _CORE_CACHE[ck] = _make_core_runner(ck, sl, devices[c])
        runners.append(_CORE_CACHE[ck])

    q = query.reshape(S, H, D)  # B==1
    kf = key.reshape(S, KV, D)
    vf = value.reshape(S, KV, D)

    # dispatch all cores (async), then gather
    pending = []
    for c in range(KV):
        run, meta, sl = runners[c]
        plans, bias_idx, mask_idx, NM, nb = meta
        q_heads = np.ascontiguousarray(
            q[:, c * G : (c + 1) * G, :].transpose(1, 0, 2)
        )
        hi = _build_host_inputs(
            q_heads, kf[:, c, :], vf[:, c, :], sl, plans, bias_idx, mask_idx, NM, nb
        )
        pending.append(run(hi))

    out = np.empty((B, S, H * D), np.float32)
    for c in range(KV):
        oc = np.asarray(pending[c]["out"])  # [G, S, D]
        for g in range(G):
            h = c * G + g
            out[0, :, h * D : (h + 1) * D] = oc[g]
    return out
